# revision 2
# baseline (speedup 1.0000x reference)
"""Trainium2 Bass kernel for the H2+ ion PINN loss (nn_NN_ion_52347061403910).

Same math as baseline kernel (psi via 2-input base MLP; Laplacian via
gradient+Hessian heads; E(R)/decay(R) via runtime-fitted Chebyshev), with:
  - all matmuls + FT intermediates in float16 (1 cyc/row on PE, 2x DVE)
  - pB/pC merged into single 128-contraction block-diag matmuls
  - pD realigned so d1/d2 land pb-major (ALL 128 rows) -> elementwise
    partners partition-align; K-form Hessian products (5 ops vs 10)
  - sigma' sign-folded into host weights: SP1n=(H1-1)*H1 needs no Square op
  - single full-width PW phase ([128,1024]) to batch sqrt/exp act-table use
  - engine rebalance: Act=sigmoid+copies, DVE=ts/stt+PSUM muls, Pool=SBUF muls

Layout: 8 cores data-parallel, 125000 pts/core padded to 128*1024.
"""

import numpy as np
from contextlib import ExitStack

import concourse.bass as bass
from concourse import bacc
import concourse.tile as tile
import concourse.mybir as mybir
from concourse.bass_utils import run_bass_kernel_spmd

F32 = mybir.dt.float32
F16 = mybir.dt.float16
AT = mybir.ActivationFunctionType
OP = mybir.AluOpType

N_CORES = 8
N_TOTAL = 1_000_000
PER_CORE = N_TOTAL // N_CORES  # 125000
NROWS = 128
NF = 1024                      # total free dim per core (padded pts = 128*NF)
CHUNK = 512                    # FT column chunk (one PSUM bank of fp32)
PADDED = NROWS * NF            # 131072
DEG_E = 10
DEG_D = 10
CHEB_COLS = 3 + (DEG_E + 1) + (DEG_D + 1)


def _sigmoid(x):
    return 1.0 / (1.0 + np.exp(-x))


def _cheb_coeffs(f, lo, hi, deg):
    k = np.arange(deg + 1)
    tn = np.cos((2 * k + 1) * np.pi / (2 * (deg + 1)))
    y = f(0.5 * (tn + 1) * (hi - lo) + lo)
    c = np.polynomial.chebyshev.chebfit(tn, y, deg)
    return np.polynomial.chebyshev.cheb2poly(c)  # power basis in t = a*R+b


def build_consts(params):
    """Host-side derived weight tensors (block-packed lhsT's), float32.

    Row layout everywhere: 32*pb + 16*br + j  (pb point-in-band, br branch).
    Sign folds: SP1n = -sigma1', SP2n = -sigma2' (computed as (H-1)*H);
    B2D1/B2D2/WCF carry a -1 so pD1=+d1, pD2=+d2, pC=+ubar;
    WOUTN = -Wo so UT = (SP2n*WOUTN)*T2 = wout*sigma2'';
    D2/D3 carry -1 because VT2 = pC*SP1n = -ubar*sigma1'.
    """
    p64 = {k: np.asarray(v, np.float64) for k, v in params.items()}
    W1 = p64["W_H1"]          # [16,2]
    W2 = p64["W_H2"]          # [16,16]
    Wo = p64["W_out"][0]      # [16]
    w0, w1 = W1[:, 0], W1[:, 1]

    def blkdiag8(blocks):
        M = np.zeros((128, 128))
        for i, B in enumerate(blocks):
            M[16 * i:16 * i + 16, 16 * i:16 * i + 16] = B
        return M

    WA = np.zeros((8, 128))
    for pb in range(4):
        for br in range(2):
            c = 32 * pb + 16 * br
            wa, wb = (w0, w1) if br == 0 else (w1, w0)
            WA[pb, c:c + 16] = wa
            WA[4 + pb, c:c + 16] = wb

    WB1F = blkdiag8([W2.T] * 8)
    WCF = blkdiag8([-(Wo[:, None] * W2)] * 8)
    bd1, bd2 = [], []
    for pb in range(4):
        for br in range(2):
            wa, wb = (w0, w1) if br == 0 else (w1, w0)
            bd1.append(-(W2 * wa[None, :]).T)
            bd2.append(-(W2 * wb[None, :]).T)
    B2D1 = blkdiag8(bd1)
    B2D2 = blkdiag8(bd2)

    D1 = np.zeros((128, 24))
    D2 = np.zeros((128, 24))
    D3 = np.zeros((128, 24))
    D4 = np.zeros((128, 24))
    D5 = np.zeros((128, 24))
    D6 = np.zeros((128, 24))
    for pb in range(4):
        for br in range(2):
            r = 32 * pb + 16 * br
            wa, wb = (w0, w1) if br == 0 else (w1, w0)
            D1[r:r + 16, 0 + pb] = Wo
            D2[r:r + 16, 4 + pb] = -wa
            D2[r:r + 16, 8 + pb] = -wb
            D3[r:r + 16, 12 + pb] = -wa * wa
            D3[r:r + 16, 16 + pb] = -wa * wb
            D3[r:r + 16, 20 + pb] = -wb * wb
        D4[32 * pb:32 * pb + 32, 12 + pb] = 1.0
        D5[32 * pb:32 * pb + 32, 20 + pb] = 1.0
        D6[32 * pb:32 * pb + 32, 16 + pb] = 1.0

    bh1 = np.tile(np.asarray(params["b_H1"], np.float64), 8)[:, None]
    bh2 = np.tile(np.asarray(params["b_H2"], np.float64), 8)[:, None]
    woutn = np.tile(-Wo, 8)[:, None]

    consts16 = dict(WA=WA, WB1F=WB1F, WCF=WCF, B2D1=B2D1, B2D2=B2D2,
                    WD1=D1, WD2=D2, WD3=D3, WD4=D4, WD5=D5, WD6=D6)
    out = {k: np.ascontiguousarray(v, np.float16) for k, v in consts16.items()}
    for k, v in dict(BH1=bh1, BH2=bh2, WOUTN=woutn).items():
        out[k] = np.ascontiguousarray(v, np.float32)
    return out


def build_cheb(params, R):
    """[128, CHEB_COLS] tile: cols [alpha, beta, b_out, cE..., cD...]."""
    p64 = {k: np.asarray(v, np.float64) for k, v in params.items()}

    def E_of_R(r):
        e = _sigmoid(np.outer(r, p64["W_E1"][:, 0]) + p64["b_E1"])
        e = _sigmoid(e @ p64["W_E2"].T + p64["b_E2"])
        return e @ p64["W_Eout"][0] + p64["b_Eout"][0]

    def D_of_R(r):
        fd = _sigmoid(np.outer(r, p64["W_DL"][:, 0]) + p64["b_DL"])
        return fd @ p64["W_D"][0] + p64["b_D"][0]

    lo = float(np.min(R)) - 1e-6
    hi = float(np.max(R)) + 1e-6
    alpha = 2.0 / (hi - lo)
    beta = -(hi + lo) / (hi - lo)
    cE = _cheb_coeffs(E_of_R, lo, hi, DEG_E)
    cD = _cheb_coeffs(D_of_R, lo, hi, DEG_D)
    row = np.concatenate([[alpha, beta, float(p64["b_out"][0])], cE, cD])
    return np.ascontiguousarray(np.tile(row[None, :], (128, 1)), np.float32)


WEIGHT_SHAPES = dict(WA=(8, 128), WB1F=(128, 128), WCF=(128, 128),
                     B2D1=(128, 128), B2D2=(128, 128),
                     WD1=(128, 24), WD2=(128, 24), WD3=(128, 24),
                     WD4=(128, 24), WD5=(128, 24), WD6=(128, 24),
                     BH1=(128, 1), BH2=(128, 1), WOUTN=(128, 1))


def build_bass(nf=NF, chunk=CHUNK):
    npc = min(chunk, nf)
    nchunks = nf // npc
    assert nf % npc == 0

    nc = bacc.Bacc("TRN2", target_bir_lowering=False, debug=False)

    X = nc.dram_tensor("X", [NROWS, nf], F16, kind="ExternalInput")
    Y = nc.dram_tensor("Y", [NROWS, nf], F16, kind="ExternalInput")
    Z = nc.dram_tensor("Z", [NROWS, nf], F16, kind="ExternalInput")
    RT = nc.dram_tensor("RT", [NROWS, nf], F16, kind="ExternalInput")
    CHEB = nc.dram_tensor("CHEB", [NROWS, CHEB_COLS], F32, kind="ExternalInput")
    MROWS = nc.dram_tensor("MROWS", [6, NF], F32, kind="ExternalInput")
    W_DT = {nm: (F32 if nm in ("BH1", "BH2", "WOUTN") else F16)
            for nm in WEIGHT_SHAPES}
    Wd = {nm: nc.dram_tensor(nm, list(shp), W_DT[nm], kind="ExternalInput")
          for nm, shp in WEIGHT_SHAPES.items()}
    PSI_D = nc.dram_tensor("PSI", [NROWS, nf], F16, kind="ExternalOutput")
    ACC_D = nc.dram_tensor("ACC", [NROWS, 1], F32, kind="ExternalOutput")

    v = nc.vector
    a = nc.scalar
    g = nc.gpsimd
    te = nc.tensor
    dma = nc.sync

    with tile.TileContext(nc) as tc, ExitStack() as ctx:
        cpool = ctx.enter_context(tc.tile_pool(name="consts", bufs=1))
        pw = ctx.enter_context(tc.tile_pool(name="pw", bufs=1))
        ft = ctx.enter_context(tc.tile_pool(name="ft", bufs=2))
        psA = ctx.enter_context(tc.tile_pool(name="psA", bufs=2, space="PSUM"))
        psB = ctx.enter_context(tc.tile_pool(name="psB", bufs=1, space="PSUM"))
        psC = ctx.enter_context(tc.tile_pool(name="psC", bufs=1, space="PSUM"))
        psD0 = ctx.enter_context(tc.tile_pool(name="psD0", bufs=1, space="PSUM"))
        psD1 = ctx.enter_context(tc.tile_pool(name="psD1", bufs=1, space="PSUM"))
        psH = ctx.enter_context(tc.tile_pool(name="psH", bufs=2, space="PSUM"))

        W = {}
        for nm in Wd:
            W[nm] = cpool.tile(list(WEIGHT_SHAPES[nm]), W_DT[nm],
                               name=f"w_{nm}", tag=f"w_{nm}")
            dma.dma_start(W[nm][:], Wd[nm][:])
        CH = cpool.tile([NROWS, CHEB_COLS], F32, name="cheb", tag="cheb")
        dma.dma_start(CH[:], CHEB[:])

        def chb(i):
            return CH[:, i:i + 1]

        def pwt(tag, name=None):
            return pw.tile([NROWS, nf], F32, name=name or tag, tag=tag)

        # ---------------- PW geometry (full width) ----------------
        # Manual buffer aliasing to fit SBUF: transients share a small tag set.
        Xt = pw.tile([NROWS, nf], F16, name="bX", tag="bX")
        Yt = pw.tile([NROWS, nf], F16, name="bY", tag="bY")
        Zt = pw.tile([NROWS, nf], F16, name="bZ", tag="bZ")
        Rt = pw.tile([NROWS, nf], F16, name="bR", tag="bR")
        Mt = pwt("bM")
        dma.dma_start(Xt[:], X[:])
        dma.dma_start(Yt[:], Y[:])
        dma.dma_start(Zt[:], Z[:])
        dma.dma_start(Rt[:], RT[:])
        # mask: valid flat idx < 125000 = 122 full rows + 72 cols of row
        # 122. memset must start at partition 0, so the 6-row tail pattern
        # comes in as a tiny (24KB) input DMA'd over rows 122..127.
        v.memset(Mt[:], 1.0)
        dma.dma_start(Mt[122:128, :], MROWS[:])

        T1t, T2t, Sc, Sd = (pwt(t) for t in ("bT1", "bT2", "bSc", "bSd"))
        YZ2t, R1t, R2t = pwt("bYZ2"), pwt("bR1"), pwt("bR2")
        g.tensor_sub(T1t[:], Xt[:], Rt[:])        # D1 = x - R
        g.tensor_add(T2t[:], Xt[:], Rt[:])        # D2 = x + R
        a.square(Sc[:], Yt[:])
        a.square(Sd[:], Zt[:])
        g.tensor_add(YZ2t[:], Sc[:], Sd[:])       # YZ2
        a.square(Sc[:], T1t[:])
        g.tensor_add(R1t[:], Sc[:], YZ2t[:])      # S1
        a.square(Sd[:], T2t[:])
        g.tensor_add(R2t[:], Sd[:], YZ2t[:])      # S2
        a.sqrt(R1t[:], R1t[:])                    # R1 (in-place)
        a.sqrt(R2t[:], R2t[:])                    # R2
        Q1t, Q2t = pwt("bQ1"), pwt("bQ2")
        v.reciprocal_approx_accurate(Q1t[:], R1t[:], scratch=Sc[:])
        v.reciprocal_approx_accurate(Q2t[:], R2t[:], scratch=Sd[:])
        F1t, F2t = pwt("bF1"), pwt("bF2")
        a.activation(F1t[:], R1t[:], AT.Exp, scale=-1.0)
        a.activation(F2t[:], R2t[:], AT.Exp, scale=-1.0)  # R1t, R2t free
        POTt = pwt("bPOT")
        g.tensor_add(POTt[:], Q1t[:], Q2t[:])
        F1SQ, F2SQ = pwt("bF1SQ"), pwt("bF2SQ")
        a.square(F1SQ[:], F1t[:])
        a.square(F2SQ[:], F2t[:])
        P12t = pwt("bP12")
        g.tensor_mul(T1t[:], T1t[:], T2t[:])      # D1*D2 (in-place)
        g.tensor_add(T1t[:], T1t[:], YZ2t[:])     # + YZ2
        v.tensor_mul(T2t[:], Q1t[:], Q2t[:])
        g.tensor_mul(T2t[:], T2t[:], T1t[:])
        v.tensor_mul(T1t[:], F1t[:], F2t[:])
        g.tensor_mul(P12t[:], T1t[:], T2t[:])
        S1L, S2L = pwt("bS1L"), pwt("bS2L")
        v.tensor_scalar(T1t[:], Q1t[:], -2.0, 1.0, OP.mult, OP.add)
        g.tensor_mul(S1L[:], T1t[:], F1t[:])
        v.tensor_scalar(T2t[:], Q2t[:], -2.0, 1.0, OP.mult, OP.add)
        g.tensor_mul(S2L[:], T2t[:], F2t[:])      # Q1t, Q2t free

        # ---- Chebyshev E(R), decay(R): E-Horner on DVE, D-Horner on Pool
        RNt = pwt("bRN")
        v.tensor_scalar(RNt[:], Rt[:], chb(0), chb(1), OP.mult, OP.add)  # RN
        EEt, DECt = pwt("bEE"), pwt("bDEC")

        def horner(eng, out, base, deg):
            eng.tensor_scalar_mul(out[:], RNt[:], chb(base + deg))
            for k in range(deg - 1, 0, -1):
                eng.scalar_tensor_tensor(out[:], out[:], chb(base + k),
                                         RNt[:], OP.add, OP.mult)
            eng.tensor_scalar_add(out[:], out[:], chb(base))

        horner(v, EEt, 3, DEG_E)
        horner(v, DECt, 3 + DEG_E + 1, DEG_D)

        F1h = pw.tile([NROWS, nf], F16, name="F1h", tag="F1h")
        F2h = pw.tile([NROWS, nf], F16, name="F2h", tag="F2h")
        v.tensor_copy(F1h[:], F1t[:])
        v.tensor_copy(F2h[:], F2t[:])

        Gt, G1t, G2t = pwt("Gt"), pwt("G1t"), pwt("G2t")
        H11t, H12t, H22t = pwt("H11t"), pwt("H12t"), pwt("H22t")
        heads = [Gt, G1t, G2t, H11t, H12t, H22t]

        # ---------------- FT phase: 2 chunks x 32 bands ----------------
        for c in range(nchunks):
            cs = slice(npc * c, npc * c + npc)
            for b in range(NROWS // 4):
                rows = slice(4 * b, 4 * b + 4)
                rhsA = ft.tile([8, npc], F16, name="rhsA", tag="rhsA")
                dma.dma_start(rhsA[0:4, :], F1h[rows, cs])
                dma.dma_start(rhsA[4:8, :], F2h[rows, cs])
                pA = psA.tile([128, npc], F32, name="pA", tag="pA")
                te.matmul(pA[:], W["WA"][:], rhsA[:],
                          start=True, stop=True)
                H1 = ft.tile([128, npc], F16, name="H1", tag="H1")
                a.activation(H1[:], pA[:], AT.Sigmoid, bias=W["BH1"][:, 0:1])
                SP1n = ft.tile([128, npc], F16, name="SP1n", tag="SP1n")
                v.scalar_tensor_tensor(SP1n[:], H1[:], 1.0, H1[:],
                                       OP.subtract, OP.mult)
                T1 = ft.tile([128, npc], F16, name="T1", tag="T1")
                v.tensor_scalar(T1[:], H1[:], -2.0, 1.0, OP.mult, OP.add)

                pB = psB.tile([128, npc], F32, name="pB", tag="pB")
                te.matmul(pB[:], W["WB1F"][:], H1[:],
                          start=True, stop=True)
                H2 = ft.tile([128, npc], F16, name="H2", tag="H2")
                a.activation(H2[:], pB[:], AT.Sigmoid, bias=W["BH2"][:, 0:1])
                SP2n = ft.tile([128, npc], F16, name="SP2n", tag="SP2n")
                v.scalar_tensor_tensor(SP2n[:], H2[:], 1.0, H2[:],
                                       OP.subtract, OP.mult)
                T2 = ft.tile([128, npc], F16, name="T2", tag="T2")
                v.tensor_scalar(T2[:], H2[:], -2.0, 1.0, OP.mult, OP.add)

                pC = psC.tile([128, npc], F32, name="pC", tag="pC")
                te.matmul(pC[:], W["WCF"][:], SP2n[:],
                          start=True, stop=True)
                pD1 = psD0.tile([128, npc], F32, name="pD1", tag="pD1")
                te.matmul(pD1[:], W["B2D1"][:], SP1n[:],
                          start=True, stop=True)
                pD2 = psD1.tile([128, npc], F32, name="pD2", tag="pD2")
                te.matmul(pD2[:], W["B2D2"][:], SP1n[:],
                          start=True, stop=True)

                UT = ft.tile([128, npc], F16, name="UT", tag="UT")
                v.scalar_tensor_tensor(UT[:], SP2n[:], W["WOUTN"][:, 0:1],
                                       T2[:], OP.mult, OP.mult)
                VT2 = ft.tile([128, npc], F16, name="VT2", tag="VT2")
                v.tensor_mul(VT2[:], pC[:], SP1n[:])
                VT = ft.tile([128, npc], F16, name="VT", tag="VT")
                v.tensor_mul(VT[:], VT2[:], T1[:])
                Dc1 = ft.tile([128, npc], F16, name="Dc1", tag="Dc1")
                a.copy(Dc1[:], pD1[:])
                Dc2 = ft.tile([128, npc], F16, name="Dc2", tag="Dc2")
                a.copy(Dc2[:], pD2[:])
                K1 = ft.tile([128, npc], F16, name="K1", tag="K1")
                g.tensor_mul(K1[:], UT[:], Dc1[:])
                K2 = ft.tile([128, npc], F16, name="K2", tag="K2")
                g.tensor_mul(K2[:], UT[:], Dc2[:])
                HSQ1 = ft.tile([128, npc], F16, name="HSQ1", tag="HSQ1")
                g.tensor_mul(HSQ1[:], K1[:], Dc1[:])
                HSQ2 = ft.tile([128, npc], F16, name="HSQ2", tag="HSQ2")
                g.tensor_mul(HSQ2[:], K2[:], Dc2[:])
                HXT = ft.tile([128, npc], F16, name="HXT", tag="HXT")
                g.tensor_mul(HXT[:], K1[:], Dc2[:])

                pH = psH.tile([24, npc], F32, name="pH", tag="pH")
                te.matmul(pH[:], W["WD1"][:], H2[:],
                          start=True, stop=False)
                te.matmul(pH[:], W["WD2"][:], VT2[:],
                          start=False, stop=False)
                te.matmul(pH[:], W["WD3"][:], VT[:],
                          start=False, stop=False)
                te.matmul(pH[:], W["WD4"][:], HSQ1[:],
                          start=False, stop=False)
                te.matmul(pH[:], W["WD5"][:], HSQ2[:],
                          start=False, stop=False)
                te.matmul(pH[:], W["WD6"][:], HXT[:],
                          start=False, stop=True)
                HST = ft.tile([24, npc], F32, name="HST", tag="HST")
                a.copy(HST[:], pH[:])
                for i, ht in enumerate(heads):
                    dma.dma_start(ht[rows, cs], HST[4 * i:4 * i + 4, :])

        # ---------------- assembly (full width, reuses freed PW tags) ------
        t_a = pwt("bT1", name="t_a")
        t_b = pwt("bT2", name="t_b")
        PSIt = pwt("bQ1", name="PSIt")
        v.scalar_tensor_tensor(PSIt[:], Gt[:], chb(2), DECt[:],
                               OP.add, OP.mult)
        g.tensor_add(PSIt[:], PSIt[:], F1t[:])
        g.tensor_add(PSIt[:], PSIt[:], F2t[:])
        PSIh = pw.tile([NROWS, nf], F16, name="PSIh", tag="PSIh")
        v.tensor_copy(PSIh[:], PSIt[:])
        dma.dma_start(PSI_D[:], PSIh[:])
        LAPG = pwt("bQ2", name="LAPG")
        v.tensor_mul(t_a[:], G1t[:], S1L[:])
        g.tensor_mul(t_b[:], G2t[:], S2L[:])
        g.tensor_add(LAPG[:], t_a[:], t_b[:])
        g.tensor_mul(t_a[:], H11t[:], F1SQ[:])
        g.tensor_add(LAPG[:], LAPG[:], t_a[:])
        v.scalar_tensor_tensor(t_b[:], P12t[:], 2.0, H12t[:],
                               OP.mult, OP.mult)
        g.tensor_add(LAPG[:], LAPG[:], t_b[:])
        v.tensor_mul(t_a[:], H22t[:], F2SQ[:])
        g.tensor_add(LAPG[:], LAPG[:], t_a[:])
        v.tensor_mul(LAPG[:], DECt[:], LAPG[:])   # in-place: *decay
        g.tensor_add(LAPG[:], LAPG[:], S1L[:])
        g.tensor_add(LAPG[:], LAPG[:], S2L[:])    # LAPG = full laplacian
        g.tensor_add(POTt[:], POTt[:], EEt[:])    # POT+E (in-place)
        REST = pwt("bR1", name="REST")
        v.tensor_mul(t_a[:], POTt[:], PSIt[:])
        v.scalar_tensor_tensor(REST[:], LAPG[:], -0.5, t_a[:],
                               OP.mult, OP.subtract)
        g.tensor_mul(REST[:], REST[:], Mt[:])
        acc = cpool.tile([NROWS, 1], F32, name="acc", tag="acc")
        sqout = pwt("bR2", name="sqout")
        a.activation(sqout[:], REST[:], AT.Square, accum_out=acc[:])
        dma.dma_start(ACC_D[:], acc[:])

    nc.compile()
    return nc


def make_in_maps(inputs, nf=NF, n_cores=N_CORES, per_core=PER_CORE):
    """Build the per-core input maps (shards + shared consts)."""
    params = {k: v for k, v in inputs.items() if k not in
              ("x", "y", "z", "R", "bIndex1", "bIndex2")}
    consts = build_consts(params)
    cheb = build_cheb(params, np.asarray(inputs["R"], np.float32))
    padded = NROWS * nf
    mtail = np.zeros((6, nf), np.float32)
    mtail[0, :per_core - 122 * nf] = 1.0

    in_maps = []
    for core in range(n_cores):
        sl = slice(core * per_core, (core + 1) * per_core)

        def shard(arr, fill):
            s = np.asarray(arr, np.float32)[sl, 0]
            out = np.full(padded, fill, np.float32)
            out[:s.shape[0]] = s
            return out.reshape(NROWS, nf).astype(np.float16)

        m = dict(consts)
        m["MROWS"] = mtail
        m["X"] = shard(inputs["x"], 0.5)
        m["Y"] = shard(inputs["y"], 0.5)
        m["Z"] = shard(inputs["z"], 0.5)
        m["RT"] = shard(inputs["R"], 1.0)
        m["CHEB"] = cheb
        in_maps.append(m)
    return in_maps


_NC_CACHE = {}


def kernel(**inputs):
    if "nc" not in _NC_CACHE:
        _NC_CACHE["nc"] = build_bass()
    nc = _NC_CACHE["nc"]

    in_maps = make_in_maps(inputs)
    results = run_bass_kernel_spmd(nc, in_maps, core_ids=list(range(N_CORES)))
    outs = results.results

    psi = np.concatenate([np.asarray(outs[c]["PSI"], np.float32)
                          .reshape(-1)[:PER_CORE] for c in range(N_CORES)])
    res2 = float(sum(np.asarray(outs[c]["ACC"], np.float64).sum()
                     for c in range(N_CORES)))
    b1 = np.asarray(inputs["bIndex1"]).astype(np.int64)
    b2 = np.asarray(inputs["bIndex2"]).astype(np.int64)
    psi64 = psi.astype(np.float64)
    loss = res2 / N_TOTAL + (psi64[b1] ** 2).mean() + (psi64[b2] ** 2).mean()
    return np.float32(loss)
